# revision 1
# baseline (speedup 1.0000x reference)
"""Multi-head self-attention (B=8, T=2048, C=192, H=6, HS=32) on 8 TRN2 cores.

Sharding: data-parallel over batch — core i computes batch element i fully
on-chip (no collectives). Host pre-transposes x and packs weights so the
device does zero transposes:

  qT/kT [d, t] = Wq_packed.T @ x.T          (d = h*HS + dd)
  v     [s, d] = x @ Wv_packed, stored per-head as [v_h | ones] (33 cols)
  S^T   [s, t] = kT_h.T @ qT_h              (K=32 matmuls, row-group packed)
  P^T          = exp(S^T / sqrt(HS))        (ScalarE, PSUM->SBUF, bf16)
  [O^T_h; rowsum_h x32] = [v_h|1x32].T @ P^T  (rowsum replicated to rows 32-63)
  OTn_h [d, t] = O^T_h * (1/rowsum_h)       (DVE reciprocal + mul, no bcast)
  out   [t, c] = sum_h OTn_h.T @ Wproj_h + bias  (K=32 accum + rank-1 bias)
"""

import numpy as np
import ml_dtypes
from contextlib import ExitStack

import concourse.bass as bass
import concourse.tile as tile
from concourse import bacc, mybir
from concourse.bass_utils import run_bass_kernel_spmd

B, T, C = 8, 2048, 192
H, HS = 6, 32
P = 128
TCH = 512            # t-chunk width (one PSUM bank of fp32)
NT = T // TCH        # 4
NS = T // P          # 16 s-tiles
SCALE = 1.0 / float(np.sqrt(HS))
BF16 = mybir.dt.bfloat16
F32 = mybir.dt.float32
Exp = mybir.ActivationFunctionType.Exp

_CACHE = {}


def build_nc():
    nc = bacc.Bacc()
    xT = nc.declare_dram_parameter("xT", [C, T], BF16, isOutput=False)
    wq = nc.declare_dram_parameter("wq", [C, H * HS], BF16, isOutput=False)
    wk = nc.declare_dram_parameter("wk", [C, H * HS], BF16, isOutput=False)
    wv = nc.declare_dram_parameter("wv", [C, H * HS], BF16, isOutput=False)
    wp = nc.declare_dram_parameter("wp", [H, HS, C], BF16, isOutput=False)
    bp = nc.declare_dram_parameter("bp", [1, C], BF16, isOutput=False)
    out = nc.declare_dram_parameter("out", [T, C], F32, isOutput=True)

    with tile.TileContext(nc) as tc, ExitStack() as ctx:
        singles = ctx.enter_context(tc.tile_pool(name="singles", bufs=1))
        qk_pool = ctx.enter_context(tc.tile_pool(name="qk", bufs=1))
        vaug_pool = ctx.enter_context(tc.tile_pool(name="vaug", bufs=1))
        pt_pool = ctx.enter_context(tc.tile_pool(name="ptp", bufs=4))
        otn_pool = ctx.enter_context(tc.tile_pool(name="otn", bufs=1))
        small = ctx.enter_context(tc.tile_pool(name="small", bufs=4))
        ysb_pool = ctx.enter_context(tc.tile_pool(name="ysb", bufs=3))

        # ---------------- load inputs ----------------
        xT_a = singles.tile([P, T], BF16)
        nc.sync.dma_start(xT_a, xT[0:P, :])
        xT_b = singles.tile([C - P, T], BF16)
        nc.sync.dma_start(xT_b, xT[P:C, :])

        w_sb = {}
        for name, dram in (("q", wq), ("k", wk), ("v", wv)):
            a = singles.tile([P, H * HS], BF16, name=f"w{name}a")
            nc.sync.dma_start(a, dram[0:P, :])
            b = singles.tile([C - P, H * HS], BF16, name=f"w{name}b")
            nc.sync.dma_start(b, dram[P:C, :])
            w_sb[name] = (a, b)

        wp_sb = []
        for h in range(H):
            wph = singles.tile([HS, C], BF16, name=f"wp{h}")
            nc.sync.dma_start(wph, wp[h, :, :])
            wp_sb.append(wph)
        bp_sb = singles.tile([1, C], BF16)
        nc.sync.dma_start(bp_sb, bp[:, :])
        ones1 = singles.tile([1, P], BF16)
        nc.vector.memset(ones1, 1.0)

        # ---------------- phase 1: qT, kT, v_aug ----------------
        qT_a = qk_pool.tile([P, T], BF16)       # heads 0..3, d-major
        qT_b = qk_pool.tile([C - P, T], BF16)   # heads 4,5
        kT_a = qk_pool.tile([P, T], BF16)
        kT_b = qk_pool.tile([C - P, T], BF16)
        v_aug = []
        with tc.tile_pool(name="pqkv", bufs=2, space="PSUM") as pqkv:
            for proj, dst_a, dst_b in (("q", qT_a, qT_b), ("k", kT_a, kT_b)):
                wa, wb = w_sb[proj]
                for dlo, dsz, dst in ((0, P, dst_a), (P, C - P, dst_b)):
                    for t0 in range(0, T, TCH):
                        ps = pqkv.tile([P, TCH], F32, name="psq", tag="psq")
                        nc.tensor.matmul(
                            ps[0:dsz, :], wa[:, dlo:dlo + dsz],
                            xT_a[:, t0:t0 + TCH], start=True, stop=False)
                        nc.tensor.matmul(
                            ps[0:dsz, :], wb[:, dlo:dlo + dsz],
                            xT_b[:, t0:t0 + TCH], start=False, stop=True)
                        nc.vector.tensor_copy(
                            dst[0:dsz, t0:t0 + TCH], ps[0:dsz, :])
            wva, wvb = w_sb["v"]
            for si in range(NS):
                s0 = si * P
                va = vaug_pool.tile(
                    [P, H * 2 * HS], BF16, name=f"vaug{si}", tag=f"vaug{si}")
                ps = pqkv.tile([P, H * HS], F32, name="psv", tag="psv")
                nc.tensor.matmul(ps, xT_a[:, s0:s0 + P], wva,
                                 start=True, stop=False)
                nc.tensor.matmul(ps, xT_b[:, s0:s0 + P], wvb,
                                 start=False, stop=True)
                va_r = va.rearrange("p (h e) -> p h e", h=H)
                ps_r = ps.rearrange("p (h d) -> p h d", h=H)
                nc.vector.tensor_copy(va_r[:, :, 0:HS], ps_r)
                nc.vector.memset(va_r[:, :, HS:2 * HS], 1.0)
                v_aug.append(va)

        # ---------------- phase 2: attention ----------------
        otn = [otn_pool.tile([HS, T], BF16, name=f"otn{h}", tag=f"otn{h}")
               for h in range(H)]
        # head pairs (A=2p, B=2p+1); within a pair kT/qT rows sit in
        # distinct 32-row groups, so the two QKT matmuls run concurrently
        def hsrc(h):
            if h < 4:
                return kT_a, qT_a, HS * h
            return kT_b, qT_b, HS * (h - 4)
        with (
            tc.tile_pool(name="pst", bufs=2, space="PSUM") as pst_pool,
            tc.tile_pool(name="pav", bufs=1, space="PSUM") as pav_pool,
            tc.tile_pool(name="py", bufs=1, space="PSUM") as py_pool,
        ):
            for tc0 in range(0, T, TCH):
                av = [pav_pool.tile([P, TCH], F32,
                                    name=f"avp{p}", tag=f"avp{p}")
                      for p in range(H // 2)]
                for si in range(NS):
                    s0 = si * P
                    for p in range(H // 2):
                        hA, hB = 2 * p, 2 * p + 1
                        stp = pst_pool.tile([P, 2 * TCH], F32,
                                            name="stp", tag="stp")
                        for half, h in ((0, hA), (1, hB)):
                            kT_t, qT_t, pb = hsrc(h)
                            nc.tensor.matmul(
                                stp[:, half * TCH:(half + 1) * TCH],
                                kT_t[pb:pb + HS, s0:s0 + P],
                                qT_t[pb:pb + HS, tc0:tc0 + TCH],
                                start=True, stop=True, tile_position=(pb, 0))
                        ptp = pt_pool.tile([P, 2 * TCH], BF16,
                                           name="ptp", tag="ptp")
                        nc.scalar.activation(ptp, stp, Exp, scale=SCALE)
                        for half, h in ((0, hA), (1, hB)):
                            nc.tensor.matmul(
                                av[p][64 * half:64 * half + 64, :],
                                v_aug[si][:, 2 * HS * h:2 * HS * (h + 1)],
                                ptp[:, half * TCH:(half + 1) * TCH],
                                start=(si == 0), stop=(si == NS - 1),
                                skip_group_check=True,
                                tile_position=(0, 64 * half))
                for p in range(H // 2):
                    rbp = small.tile([P, TCH], F32, name="rbp", tag="rbp")
                    for half, h in ((0, 2 * p), (1, 2 * p + 1)):
                        b = 64 * half
                        nc.vector.reciprocal(
                            rbp[b:b + HS, :], av[p][b + HS:b + 2 * HS, :])
                        nc.vector.tensor_mul(
                            otn[h][:, tc0:tc0 + TCH],
                            av[p][b:b + HS, :], rbp[b:b + HS, :])
                # ---- projection for this t-chunk (spare PSUM bank) ----
                for tt in range(tc0, tc0 + TCH, P):
                    ps = py_pool.tile([P, C], F32, name="psy", tag="psy")
                    nc.tensor.matmul(ps, ones1, bp_sb, start=True, stop=False)
                    for h in range(H):
                        nc.tensor.matmul(
                            ps, otn[h][:, tt:tt + P], wp_sb[h],
                            start=False, stop=(h == H - 1))
                    ysb = ysb_pool.tile([P, C], F32, name="ysbt", tag="ysbt")
                    nc.vector.tensor_copy(ysb, ps)
                    nc.sync.dma_start(out[tt:tt + P, :], ysb)

    nc.compile()
    return nc


def _get_nc():
    if "nc" not in _CACHE:
        _CACHE["nc"] = build_nc()
    return _CACHE["nc"]


def make_in_maps(x, Wq, Wk, Wv, Wproj, bproj):
    bf = ml_dtypes.bfloat16
    x = np.asarray(x, np.float32)
    pack = lambda w: np.ascontiguousarray(
        np.transpose(np.asarray(w, np.float32), (1, 0, 2)).reshape(C, H * HS)
    ).astype(bf)
    wq, wk, wv = pack(Wq), pack(Wk), pack(Wv)
    wp = np.ascontiguousarray(
        np.asarray(Wproj, np.float32).reshape(H, HS, C)).astype(bf)
    bp = np.asarray(bproj, np.float32).reshape(1, C).astype(bf)
    maps = []
    for i in range(B):
        xti = np.ascontiguousarray(x[i].T).astype(bf)
        maps.append({"xT": xti, "wq": wq, "wk": wk, "wv": wv,
                     "wp": wp, "bp": bp})
    return maps


def run(inputs, trace=False, **kw):
    nc = _get_nc()
    in_maps = make_in_maps(**inputs)
    res = run_bass_kernel_spmd(nc, in_maps, core_ids=list(range(B)),
                               trace=trace, **kw)
    y = np.stack([np.asarray(res.results[i]["out"], np.float32)
                  for i in range(B)], axis=0)
    return y, res


def kernel(**inputs):
    y, _ = run(inputs, trace=False)
    return y



# revision 29
# speedup vs baseline: 1.2051x; 1.2051x over previous
"""Multi-head self-attention (B=8, T=2048, C=192, H=6, HS=32) on 8 TRN2 cores.

Sharding: data-parallel over batch - core i computes batch element i fully
on-chip (no collectives). Host pre-transposes x and packs weights.

Per core:
  qkT [384, t]  = wqk.T @ x.T, packed into 3x[128, T] tiles (q0-3 | q4,q5,k0,k1 | k2-5)
  v_aug [s, 6*33] = x @ Wv, per-head [v_h | ones] (33 cols)
  S^T [s, t]    = kT_h.T @ qT_h        (per (si, h), one 512-wide matmul)
  P^T           = exp(S^T / sqrt(HS))  split across 3 engines:
                    ScalarE: exact exp activation
                    DVE/GPSIMD: Schraudolph exp via int16 bit trick
                      bf16bits(exp(s)) ~ int16(s*AEXP + BEXP)
  O[t, d+sum]   = PV with P^T as STATIONARY (cost ~ 33/row vs 512/row)
  normalize per-partition 1/rowsum (tensor_scalar with per-partition scalar)
  O^T via PE transpose; Y[t, c] = O^T.T @ Wproj + bias; DMA out.
"""

import numpy as np
import ml_dtypes
from collections import deque
from contextlib import ExitStack

import concourse.bass as bass
import concourse.tile as tile
from concourse import bacc, mybir
from concourse.bass_utils import run_bass_kernel_spmd

B, T, C = 8, 2048, 192
H, HS = 6, 32
P = 128
TCH = 512            # t-chunk width (one PSUM bank of fp32)
NCH = T // TCH       # 4
NS = T // P          # 16 s-tiles
NJ = TCH // P        # 4 t-tiles per chunk
SCALE = 1.0 / float(np.sqrt(HS))
BF16 = mybir.dt.bfloat16
F32 = mybir.dt.float32
I16 = mybir.dt.int16
Exp = mybir.ActivationFunctionType.Exp
MUL = mybir.AluOpType.mult
ADD = mybir.AluOpType.add

# Schraudolph exp in bf16-bit domain: bf16_bits = int16(s * AEXP + BEXP)
AEXP = float(SCALE * 128.0 * np.log2(np.e))
BEXP = 16248.5

# per-chunk engine split for the 48 exp pair-tiles (GPSIMD cannot access
# PSUM, so only ScalarE (exact exp) and DVE (Schraudolph) share the work)
W_SC, W_DV = 27, 21


def _mk_pattern():
    ev = sorted(
        [((i + 0.5) / W_SC, 0) for i in range(W_SC)]
        + [((i + 0.5) / W_DV, 1) for i in range(W_DV)]
    )
    return [e for _, e in ev]


EXP_PAT = _mk_pattern()
LAG = 2  # software pipeline depth (in pair-steps) between QKT and PV

_CACHE = {}


def build_nc():
    nc = bacc.Bacc()
    xT = nc.declare_dram_parameter("xT", [C, T], BF16, isOutput=False)
    wqk = nc.declare_dram_parameter("wqk", [C, 2 * H * HS], BF16, isOutput=False)
    wv = nc.declare_dram_parameter("wv", [C, H * HS], BF16, isOutput=False)
    wp = nc.declare_dram_parameter("wp", [H * HS, C], BF16, isOutput=False)
    bp = nc.declare_dram_parameter("bp", [1, C], BF16, isOutput=False)
    out = nc.declare_dram_parameter("out", [T, C], F32, isOutput=True)

    with tile.TileContext(nc) as tc, ExitStack() as ctx:
        singles = ctx.enter_context(tc.tile_pool(name="singles", bufs=1))
        qk_pool = ctx.enter_context(tc.tile_pool(name="qk", bufs=1))
        va_pool = ctx.enter_context(tc.tile_pool(name="va", bufs=1))
        pt_pool = ctx.enter_context(tc.tile_pool(name="pt", bufs=1))
        on_pool = ctx.enter_context(tc.tile_pool(name="on", bufs=3))
        ot_pool = ctx.enter_context(tc.tile_pool(name="ot", bufs=4))
        rc_pool = ctx.enter_context(tc.tile_pool(name="rc", bufs=2))
        ysb_pool = ctx.enter_context(tc.tile_pool(name="ysb", bufs=3))
        ps_pool = ctx.enter_context(tc.tile_pool(name="ps", bufs=2, space="PSUM"))
        po_pool = ctx.enter_context(tc.tile_pool(name="po", bufs=4, space="PSUM"))

        def ps_tile(name):
            return ps_pool.tile([P, 2 * TCH], F32, name=name, tag="ps")

        # ---------------- load inputs ----------------
        xT_a = singles.tile([P, T], BF16)
        nc.sync.dma_start(xT_a, xT[0:P, :])
        xT_b = singles.tile([C - P, T], BF16)
        nc.sync.dma_start(xT_b, xT[P:C, :])
        wqk_a = singles.tile([P, 2 * H * HS], BF16)
        nc.sync.dma_start(wqk_a, wqk[0:P, :])
        wqk_b = singles.tile([C - P, 2 * H * HS], BF16)
        nc.sync.dma_start(wqk_b, wqk[P:C, :])
        wv_a = singles.tile([P, H * HS], BF16)
        nc.sync.dma_start(wv_a, wv[0:P, :])
        wv_b = singles.tile([C - P, H * HS], BF16)
        nc.sync.dma_start(wv_b, wv[P:C, :])
        wp_a = singles.tile([P, C], BF16)
        nc.sync.dma_start(wp_a, wp[0:P, :])
        # second k-slab of Wproj lives at partition base 64 so it matches the
        # base of the overlapping DMA-transpose output OTy (d rows 64..191)
        wp_bb = singles.tile([P, C], BF16)
        nc.sync.dma_start(wp_bb[64:P, :], wp[P:H * HS, :])
        bp_sb = singles.tile([1, C], BF16)
        nc.sync.dma_start(bp_sb, bp[:, :])
        ones1 = singles.tile([1, P], BF16)
        nc.vector.memset(ones1, 1.0)

        # ---------------- phase 1: qT/kT and v_aug ----------------
        # PE requires fmap/weight at the same partition base, so q_h and k_h
        # live in separate tiles at identical row offsets.
        # wqk col order: [q0-3 (128) | k0-3 (128) | q4,q5 (64) | k4,k5 (64)]
        qkt = [singles.tile([P, T], BF16, name="qA"),
               singles.tile([P, T], BF16, name="kA"),
               singles.tile([P - 64, T], BF16, name="qB"),
               singles.tile([P - 64, T], BF16, name="kB")]
        qk_cols = [(0, P), (P, P), (2 * P, 64), (2 * P + 64, 64)]
        cp_engines = [nc.scalar.copy, nc.vector.tensor_copy]
        ci = 0
        for g, (c0, csz) in enumerate(qk_cols):
            for c in range(NCH):
                ps = ps_tile("psqk")
                nc.tensor.matmul(ps[0:csz, 0:TCH], wqk_a[:, c0:c0 + csz],
                                 xT_a[:, c * TCH:(c + 1) * TCH],
                                 start=True, stop=False)
                nc.tensor.matmul(ps[0:csz, 0:TCH], wqk_b[:, c0:c0 + csz],
                                 xT_b[:, c * TCH:(c + 1) * TCH],
                                 start=False, stop=True)
                cp_engines[ci % 2](qkt[g][:, c * TCH:(c + 1) * TCH],
                                   ps[0:csz, 0:TCH])
                ci += 1

        va = [va_pool.tile([P, H * 33], BF16, name=f"va{si}") for si in range(NS)]
        for si in range(NS):
            ps = ps_tile("psv")
            nc.tensor.matmul(ps[:, 0:H * HS], xT_a[:, si * P:(si + 1) * P],
                             wv_a, start=True, stop=False)
            nc.tensor.matmul(ps[:, 0:H * HS], xT_b[:, si * P:(si + 1) * P],
                             wv_b, start=False, stop=True)
            va_r = va[si].rearrange("p (h e) -> p h e", h=H)
            ps_r = ps[:, 0:H * HS].rearrange("p (h d) -> p h d", h=H)
            (nc.vector.tensor_copy if si % 2 == 0 else nc.scalar.copy)(
                va_r[:, :, 0:HS], ps_r)
            nc.gpsimd.memset(va_r[:, :, 32], 1.0)

        # head -> (tile, row) maps; q_h and k_h share the same row base
        def qsrc(h):
            return (qkt[0], HS * h) if h < 4 else (qkt[2], HS * (h - 4))

        def ksrc(h):
            return (qkt[1], HS * h) if h < 4 else (qkt[3], HS * (h - 4))

        # ---------------- phase 2 ----------------
        # ptp[p3][si]: P^T for head pair (2*p3, 2*p3+1); halves 512 cols each
        ptp = [[pt_pool.tile([P, 2 * TCH], BF16, name=f"pt{p3}_{si}")
                for si in range(NS)] for p3 in range(3)]

        for c in range(NCH):
            po = [po_pool.tile([P, TCH], F32, name=f"po{j}", tag=f"po{j}",
                               bufs=1)
                  for j in range(NJ)]
            pairs = [(si, p3) for si in range(NS) for p3 in range(3)]
            npair = len(pairs)

            def issue_pv(pr):
                si, p3 = pairs[pr]
                for half in (0, 1):
                    h = 2 * p3 + half
                    for j in range(NJ):
                        nc.tensor.matmul(
                            po[j][:, h * 33:(h + 1) * 33],
                            ptp[p3][si][:, half * TCH + j * P:
                                        half * TCH + (j + 1) * P],
                            va[si][:, h * 33:(h + 1) * 33],
                            start=(pr == 0 and half == 0),
                            stop=(pr == npair - 1 and half == 1),
                            tile_position=(0, 0), skip_group_check=True)

            for pr, (si, p3) in enumerate(pairs):
                ps = ps_tile("psst")
                for half in (0, 1):
                    h = 2 * p3 + half
                    kt, kr = ksrc(h)
                    qt, qr = qsrc(h)
                    nc.tensor.matmul(
                        ps[:, half * TCH:(half + 1) * TCH],
                        kt[kr:kr + HS, si * P:(si + 1) * P],
                        qt[qr:qr + HS, c * TCH:(c + 1) * TCH],
                        start=True, stop=True,
                        tile_position=(kr, 0), skip_group_check=True)
                eng = EXP_PAT[pr]
                dst = ptp[p3][si]
                if eng == 0:
                    nc.scalar.activation(dst, ps, Exp, scale=SCALE)
                else:
                    nc.vector.tensor_scalar(dst[:, :].bitcast(I16), ps,
                                            AEXP, BEXP, MUL, ADD)
                if pr >= LAG:
                    issue_pv(pr - LAG)
            for pr in range(npair - LAG, npair):
                issue_pv(pr)

            # ---- stage C: normalize, DMA-transpose, project, store ----
            ons = []
            for j in range(NJ):
                po_r = po[j][:, 0:H * 33].rearrange("p (h e) -> p h e", h=H)
                rcp = rc_pool.tile([P, 8], F32, name="rcp", tag="rc")
                nc.vector.reciprocal(rcp[:, 0:H], po_r[:, :, 32])
                on = on_pool.tile([P, C], BF16, name="on", tag="on")
                on_r = on[:, :].rearrange("p (h e) -> p h e", h=H)
                nc.vector.tensor_tensor(
                    on_r, po_r[:, :, 0:HS],
                    rcp[:, 0:H].to_broadcast([P, H, HS]), MUL)
                # O^T via XBAR DMA transpose (SBUF->SBUF): d rows 0..127 and
                # (overlapping) 64..191, so both proj matmuls have matching
                # fmap/weight partition bases (0 and 64).
                otx = ot_pool.tile([P, P], BF16, name="otx", tag="otx")
                oty = ot_pool.tile([P, P], BF16, name="oty", tag="oty")
                nc.sync.dma_start_transpose(otx, on[:, 0:P])
                nc.sync.dma_start_transpose(oty, on[:, 64:64 + P])
                ons.append((on, otx, oty))
            for j in range(NJ):
                t0 = c * TCH + j * P
                on, otx, oty = ons[j]
                psy = po_pool.tile([P, TCH], F32, name=f"psy{j}",
                                   tag=f"po{j}", bufs=1)
                nc.tensor.matmul(psy[:, 0:C], otx, wp_a,
                                 start=True, stop=False)
                nc.tensor.matmul(psy[:, 0:C], oty[64:P, :], wp_bb[64:P, :],
                                 start=False, stop=False,
                                 tile_position=(64, 0),
                                 skip_group_check=True)
                nc.tensor.matmul(psy[:, 0:C], ones1, bp_sb,
                                 start=False, stop=True)
                ysb = ysb_pool.tile([P, C], F32, name="ysb", tag="y")
                (nc.scalar.copy if j % 2 == 0 else nc.vector.tensor_copy)(
                    ysb, psy[:, 0:C])
                nc.sync.dma_start(out[t0:t0 + P, :], ysb)

    nc.compile()
    return nc


def _get_nc():
    if "nc" not in _CACHE:
        _CACHE["nc"] = build_nc()
    return _CACHE["nc"]


def make_in_maps(x, Wq, Wk, Wv, Wproj, bproj):
    bf = ml_dtypes.bfloat16
    x = np.asarray(x, np.float32)
    pack = lambda w: np.ascontiguousarray(
        np.transpose(np.asarray(w, np.float32), (1, 0, 2)).reshape(C, H * HS))
    pq, pk = pack(Wq), pack(Wk)
    # col order: q0-3 | k0-3 | q4,q5 | k4,k5 (q_h/k_h at equal row bases)
    wqk = np.ascontiguousarray(np.concatenate(
        [pq[:, 0:128], pk[:, 0:128], pq[:, 128:192], pk[:, 128:192]],
        axis=1)).astype(bf)
    wv = pack(Wv).astype(bf)
    wp = np.ascontiguousarray(np.asarray(Wproj, np.float32)).astype(bf)
    bp = np.asarray(bproj, np.float32).reshape(1, C).astype(bf)
    maps = []
    for i in range(B):
        xti = np.ascontiguousarray(x[i].T).astype(bf)
        maps.append({"xT": xti, "wqk": wqk, "wv": wv, "wp": wp, "bp": bp})
    return maps


def run(inputs, trace=False, **kw):
    nc = _get_nc()
    in_maps = make_in_maps(**inputs)
    res = run_bass_kernel_spmd(nc, in_maps, core_ids=list(range(B)),
                               trace=trace, **kw)
    y = np.stack([np.asarray(res.results[i]["out"], np.float32)
                  for i in range(B)], axis=0)
    return y, res


def kernel(**inputs):
    y, _ = run(inputs, trace=False)
    return y


# revision 35
# speedup vs baseline: 1.5868x; 1.3168x over previous
"""Multi-head self-attention (B=8, T=2048, C=192, H=6, HS=32) on 8 TRN2 cores.

Sharding: data-parallel over batch - core i computes batch element i fully
on-chip (no collectives). Host pre-transposes x and packs weights.

Per core:
  qkT [384, t]  = wqk.T @ x.T, packed into 3x[128, T] tiles (q0-3 | q4,q5,k0,k1 | k2-5)
  v_aug [s, 6*33] = x @ Wv, per-head [v_h | ones] (33 cols)
  S^T [s, t]    = kT_h.T @ qT_h        (per (si, h), one 512-wide matmul)
  P^T           = exp(S^T / sqrt(HS))  split across 3 engines:
                    ScalarE: exact exp activation
                    DVE/GPSIMD: Schraudolph exp via int16 bit trick
                      bf16bits(exp(s)) ~ int16(s*AEXP + BEXP)
  O[t, d+sum]   = PV with P^T as STATIONARY (cost ~ 33/row vs 512/row)
  normalize per-partition 1/rowsum (tensor_scalar with per-partition scalar)
  O^T via PE transpose; Y[t, c] = O^T.T @ Wproj + bias; DMA out.
"""

import numpy as np
import ml_dtypes
from collections import deque
from contextlib import ExitStack

import concourse.bass as bass
import concourse.tile as tile
from concourse import bacc, mybir
from concourse.bass_utils import run_bass_kernel_spmd

B, T, C = 8, 2048, 192
H, HS = 6, 32
P = 128
TCH = 512            # t-chunk width (one PSUM bank of fp32)
NCH = T // TCH       # 4
NS = T // P          # 16 s-tiles
NJ = TCH // P        # 4 t-tiles per chunk
SCALE = 1.0 / float(np.sqrt(HS))
BF16 = mybir.dt.bfloat16
F32 = mybir.dt.float32
I16 = mybir.dt.int16
Exp = mybir.ActivationFunctionType.Exp
MUL = mybir.AluOpType.mult
ADD = mybir.AluOpType.add

# Schraudolph exp in bf16-bit domain: bf16_bits = int16(s * AEXP + BEXP)
AEXP = float(SCALE * 128.0 * np.log2(np.e))
BEXP = 16248.5

# per-chunk engine split for the 48 exp pair-tiles (GPSIMD cannot access
# PSUM, so only ScalarE (exact exp) and DVE (Schraudolph) share the work)
W_SC, W_DV = 27, 21


def _mk_pattern():
    ev = sorted(
        [((i + 0.5) / W_SC, 0) for i in range(W_SC)]
        + [((i + 0.5) / W_DV, 1) for i in range(W_DV)]
    )
    return [e for _, e in ev]


EXP_PAT = _mk_pattern()
LAG = 3  # software pipeline depth (in pair-steps) between QKT and PV

_CACHE = {}


def build_nc():
    nc = bacc.Bacc()
    xT = nc.declare_dram_parameter("xT", [C, T], BF16, isOutput=False)
    wqk = nc.declare_dram_parameter("wqk", [C, 2 * H * HS], BF16, isOutput=False)
    wv = nc.declare_dram_parameter("wv", [C, H * HS], BF16, isOutput=False)
    wp = nc.declare_dram_parameter("wp", [H * HS, C], BF16, isOutput=False)
    bp = nc.declare_dram_parameter("bp", [1, C], BF16, isOutput=False)
    out = nc.declare_dram_parameter("out", [T, C], F32, isOutput=True)

    with tile.TileContext(nc) as tc, ExitStack() as ctx:
        singles = ctx.enter_context(tc.tile_pool(name="singles", bufs=1))
        qk_pool = ctx.enter_context(tc.tile_pool(name="qk", bufs=1))
        va_pool = ctx.enter_context(tc.tile_pool(name="va", bufs=1))
        pt_pool = ctx.enter_context(tc.tile_pool(name="pt", bufs=1))
        on_pool = ctx.enter_context(tc.tile_pool(name="on", bufs=3))
        ot_pool = ctx.enter_context(tc.tile_pool(name="ot", bufs=4))
        rc_pool = ctx.enter_context(tc.tile_pool(name="rc", bufs=2))
        ysb_pool = ctx.enter_context(tc.tile_pool(name="ysb", bufs=3))
        ps_pool = ctx.enter_context(tc.tile_pool(name="ps", bufs=3, space="PSUM"))
        po_pool = ctx.enter_context(tc.tile_pool(name="po", bufs=2, space="PSUM"))

        def ps_tile(name):
            return ps_pool.tile([P, 2 * TCH], F32, name=name, tag="ps")

        # ---------------- load inputs ----------------
        xT_a = singles.tile([P, T], BF16)
        nc.sync.dma_start(xT_a, xT[0:P, :])
        xT_b = singles.tile([C - P, T], BF16)
        nc.sync.dma_start(xT_b, xT[P:C, :])
        wqk_a = singles.tile([P, 2 * H * HS], BF16)
        nc.sync.dma_start(wqk_a, wqk[0:P, :])
        wqk_b = singles.tile([C - P, 2 * H * HS], BF16)
        nc.sync.dma_start(wqk_b, wqk[P:C, :])
        wv_a = singles.tile([P, H * HS], BF16)
        nc.sync.dma_start(wv_a, wv[0:P, :])
        wv_b = singles.tile([C - P, H * HS], BF16)
        nc.sync.dma_start(wv_b, wv[P:C, :])
        wp_a = singles.tile([P, C], BF16)
        nc.sync.dma_start(wp_a, wp[0:P, :])
        # second k-slab of Wproj lives at partition base 64 so it matches the
        # base of the overlapping DMA-transpose output OTy (d rows 64..191)
        wp_bb = singles.tile([P, C], BF16)
        nc.sync.dma_start(wp_bb[64:P, :], wp[P:H * HS, :])
        bp_sb = singles.tile([1, C], BF16)
        nc.sync.dma_start(bp_sb, bp[:, :])
        ones1 = singles.tile([1, P], BF16)
        nc.vector.memset(ones1, 1.0)

        # ---------------- phase 1: qT/kT and v_aug ----------------
        # PE requires fmap/weight at the same partition base, so q_h and k_h
        # live in separate tiles at identical row offsets.
        # wqk col order: [q0-3 (128) | k0-3 (128) | q4,q5 (64) | k4,k5 (64)]
        qkt = [singles.tile([P, T], BF16, name="qA"),
               singles.tile([P, T], BF16, name="kA"),
               singles.tile([P - 64, T], BF16, name="qB"),
               singles.tile([P - 64, T], BF16, name="kB")]
        qk_cols = [(0, P), (P, P), (2 * P, 64), (2 * P + 64, 64)]
        cp_engines = [nc.scalar.copy, nc.vector.tensor_copy]
        ci = 0
        for g, (c0, csz) in enumerate(qk_cols):
            for c in range(NCH):
                ps = ps_tile("psqk")
                nc.tensor.matmul(ps[0:csz, 0:TCH], wqk_a[:, c0:c0 + csz],
                                 xT_a[:, c * TCH:(c + 1) * TCH],
                                 start=True, stop=False)
                nc.tensor.matmul(ps[0:csz, 0:TCH], wqk_b[:, c0:c0 + csz],
                                 xT_b[:, c * TCH:(c + 1) * TCH],
                                 start=False, stop=True)
                cp_engines[ci % 2](qkt[g][:, c * TCH:(c + 1) * TCH],
                                   ps[0:csz, 0:TCH])
                ci += 1

        va = [va_pool.tile([P, H * 33], BF16, name=f"va{si}") for si in range(NS)]
        for si in range(NS):
            ps = ps_tile("psv")
            nc.tensor.matmul(ps[:, 0:H * HS], xT_a[:, si * P:(si + 1) * P],
                             wv_a, start=True, stop=False)
            nc.tensor.matmul(ps[:, 0:H * HS], xT_b[:, si * P:(si + 1) * P],
                             wv_b, start=False, stop=True)
            va_r = va[si].rearrange("p (h e) -> p h e", h=H)
            ps_r = ps[:, 0:H * HS].rearrange("p (h d) -> p h d", h=H)
            (nc.vector.tensor_copy if si % 2 == 0 else nc.scalar.copy)(
                va_r[:, :, 0:HS], ps_r)
            nc.gpsimd.memset(va_r[:, :, 32], 1.0)

        # head -> (tile, row) maps; q_h and k_h share the same row base
        def qsrc(h):
            return (qkt[0], HS * h) if h < 4 else (qkt[2], HS * (h - 4))

        def ksrc(h):
            return (qkt[1], HS * h) if h < 4 else (qkt[3], HS * (h - 4))

        # ---------------- phase 2 ----------------
        # ptp[p3][si]: P^T for head pair (2*p3, 2*p3+1); halves 512 cols each
        ptp = [[pt_pool.tile([P, 2 * TCH], BF16, name=f"pt{p3}_{si}")
                for si in range(NS)] for p3 in range(3)]

        for c in range(NCH):
            # two t-tiles (j, j+1) share one PSUM bank: a single start=True
            # (first write of the bank) marks the whole bank pending-zero and
            # all later start=False writes accumulate from zero.
            po2 = [po_pool.tile([P, TCH], F32, name=f"po{jj}", tag=f"po{jj}",
                                bufs=1)
                   for jj in range(2)]
            pairs = [(si, p3) for si in range(NS) for p3 in range(3)]
            npair = len(pairs)

            def issue_pv(pr):
                si, p3 = pairs[pr]
                for half in (0, 1):
                    h = 2 * p3 + half
                    for j in range(NJ):
                        base = (j % 2) * 256
                        nc.tensor.matmul(
                            po2[j // 2][:, base + h * 33:base + (h + 1) * 33],
                            ptp[p3][si][:, half * TCH + j * P:
                                        half * TCH + (j + 1) * P],
                            va[si][:, h * 33:(h + 1) * 33],
                            start=(pr == 0 and half == 0 and j % 2 == 0),
                            stop=(pr == npair - 1 and half == 1
                                  and j % 2 == 1),
                            tile_position=(0, 0), skip_group_check=True)

            for pr, (si, p3) in enumerate(pairs):
                ps = ps_tile("psst")
                for half in (0, 1):
                    h = 2 * p3 + half
                    kt, kr = ksrc(h)
                    qt, qr = qsrc(h)
                    nc.tensor.matmul(
                        ps[:, half * TCH:(half + 1) * TCH],
                        kt[kr:kr + HS, si * P:(si + 1) * P],
                        qt[qr:qr + HS, c * TCH:(c + 1) * TCH],
                        start=True, stop=True,
                        tile_position=(kr, 0), skip_group_check=True)
                eng = EXP_PAT[pr]
                dst = ptp[p3][si]
                if eng == 0:
                    nc.scalar.activation(dst, ps, Exp, scale=SCALE)
                else:
                    nc.vector.tensor_scalar(dst[:, :].bitcast(I16), ps,
                                            AEXP, BEXP, MUL, ADD)
                if pr >= LAG:
                    issue_pv(pr - LAG)
            for pr in range(npair - LAG, npair):
                issue_pv(pr)

            # ---- stage C: normalize, DMA-transpose, project, store ----
            ons = []
            for j in range(NJ):
                base = (j % 2) * 256
                po_r = po2[j // 2][:, base:base + H * 33].rearrange(
                    "p (h e) -> p h e", h=H)
                rcp = rc_pool.tile([P, 8], F32, name="rcp", tag="rc")
                nc.vector.reciprocal(rcp[:, 0:H], po_r[:, :, 32])
                on = on_pool.tile([P, C], BF16, name="on", tag="on")
                on_r = on[:, :].rearrange("p (h e) -> p h e", h=H)
                nc.vector.tensor_tensor(
                    on_r, po_r[:, :, 0:HS],
                    rcp[:, 0:H].to_broadcast([P, H, HS]), MUL)
                # O^T via XBAR DMA transpose (SBUF->SBUF): d rows 0..127 and
                # (overlapping) 64..191, so both proj matmuls have matching
                # fmap/weight partition bases (0 and 64).
                otx = ot_pool.tile([P, P], BF16, name="otx", tag="otx")
                oty = ot_pool.tile([P, P], BF16, name="oty", tag="oty")
                nc.sync.dma_start_transpose(otx, on[:, 0:P])
                nc.sync.dma_start_transpose(oty, on[:, 64:64 + P])
                ons.append((on, otx, oty))
            psy2 = [po_pool.tile([P, TCH], F32, name=f"psy{jj}",
                                 tag=f"po{jj}", bufs=1)
                    for jj in range(2)]
            for jj in range(2):
                for j in (2 * jj, 2 * jj + 1):
                    on, otx, oty = ons[j]
                    base = (j % 2) * 256
                    psy = psy2[jj][:, base:base + C]
                    nc.tensor.matmul(psy, otx, wp_a,
                                     start=(j % 2 == 0), stop=False)
                    nc.tensor.matmul(psy, oty[64:P, :], wp_bb[64:P, :],
                                     start=False, stop=False,
                                     tile_position=(64, 0),
                                     skip_group_check=True)
                    nc.tensor.matmul(psy, ones1, bp_sb,
                                     start=False, stop=(j % 2 == 1))
                for j in (2 * jj, 2 * jj + 1):
                    t0 = c * TCH + j * P
                    base = (j % 2) * 256
                    ysb = ysb_pool.tile([P, C], F32, name="ysb", tag="y")
                    (nc.scalar.copy if j % 2 == 0 else nc.vector.tensor_copy)(
                        ysb, psy2[jj][:, base:base + C])
                    nc.sync.dma_start(out[t0:t0 + P, :], ysb)

    nc.compile()
    return nc


def _get_nc():
    if "nc" not in _CACHE:
        _CACHE["nc"] = build_nc()
    return _CACHE["nc"]


def make_in_maps(x, Wq, Wk, Wv, Wproj, bproj):
    bf = ml_dtypes.bfloat16
    x = np.asarray(x, np.float32)
    pack = lambda w: np.ascontiguousarray(
        np.transpose(np.asarray(w, np.float32), (1, 0, 2)).reshape(C, H * HS))
    pq, pk = pack(Wq), pack(Wk)
    # col order: q0-3 | k0-3 | q4,q5 | k4,k5 (q_h/k_h at equal row bases)
    wqk = np.ascontiguousarray(np.concatenate(
        [pq[:, 0:128], pk[:, 0:128], pq[:, 128:192], pk[:, 128:192]],
        axis=1)).astype(bf)
    wv = pack(Wv).astype(bf)
    wp = np.ascontiguousarray(np.asarray(Wproj, np.float32)).astype(bf)
    bp = np.asarray(bproj, np.float32).reshape(1, C).astype(bf)
    maps = []
    for i in range(B):
        xti = np.ascontiguousarray(x[i].T).astype(bf)
        maps.append({"xT": xti, "wqk": wqk, "wv": wv, "wp": wp, "bp": bp})
    return maps


def run(inputs, trace=False, **kw):
    nc = _get_nc()
    in_maps = make_in_maps(**inputs)
    res = run_bass_kernel_spmd(nc, in_maps, core_ids=list(range(B)),
                               trace=trace, **kw)
    y = np.stack([np.asarray(res.results[i]["out"], np.float32)
                  for i in range(B)], axis=0)
    return y, res


def kernel(**inputs):
    y, _ = run(inputs, trace=False)
    return y
